# revision 3
# baseline (speedup 1.0000x reference)
"""Bass/Trainium2 kernel for nn_MultiHeadedAttention (GQA + RoPE + causal attention).

Sharding: 8 cores = 2 batch groups x 4 head-groups.
Core c: batch b=c//4, head group j=c%4 (q heads 4j..4j+3, kv head j).
Output projection is column-sharded after a per-token-block AllGather of
ctx^T; host concatenates the disjoint output slices.

v2: single interleaved pipeline. Rounds t=0..3 each do
  proj(t) -> attn(qb=t) -> gather(qb=t), with outproj(qb) blocks woven in
once their gathers have landed. Scores matmuls run 2 deep ahead of the
attention*V matmuls so the exp (Scalar engine) latency never stalls the
PE, which keeps the tensor engine at its high p-state clock.
Compute is bf16 with fp32 PSUM accumulation.
"""

import os
import sys

sys.path.insert(0, "/opt/trn_rl_repo")
import numpy as np


B, S, HID = 2, 2048, 2048
NH, NKV, D = 16, 4, 128
N_CORES = 8
GROUPS = [[0, 1, 2, 3], [4, 5, 6, 7]]
HLOC = 4          # q heads per core
TB = 512          # token block (matmul moving dim)
NTB = S // TB     # 4
HT = HID // 128   # 16 hid tiles
SCALE = float(D) ** -0.5

LAST_RESULTS = None  # stash for test harness timing


def _analyze_mask(mask):
    """Per (qblock, ktile): live tiles and mixed-mask tiles (deduped)."""
    maskb = np.asarray(mask).astype(bool)
    live = []
    mixd = {}
    uniq = []
    keys = {}
    for qb in range(NTB):
        lv = []
        for kt in range(S // 128):
            sub = maskb[qb * TB:(qb + 1) * TB, kt * 128:(kt + 1) * 128]
            if not sub.any():
                continue
            lv.append(kt)
            if sub.all():
                mixd[(qb, kt)] = None
            else:
                tile = np.ascontiguousarray(sub.T.astype(np.float32))
                kb = tile.tobytes()
                if kb not in keys:
                    keys[kb] = len(uniq)
                    uniq.append(tile)
                mixd[(qb, kt)] = keys[kb]
        live.append(lv)
    return live, mixd, uniq


def _build_program(live, mixd, n_u):
    import concourse.bass as bass  # noqa: F401
    import concourse.mybir as mybir
    from concourse import bacc, tile

    f32 = mybir.dt.float32
    bf16 = mybir.dt.bfloat16
    EXP = mybir.ActivationFunctionType.Exp

    nc = bacc.Bacc("TRN2", target_bir_lowering=False, debug=False,
                   num_devices=N_CORES)

    xT = nc.dram_tensor("xT", [HID, S], bf16, kind="ExternalInput")
    wq = nc.dram_tensor("wq", [HID, HLOC * D], bf16, kind="ExternalInput")
    wk = nc.dram_tensor("wk", [HID, D], bf16, kind="ExternalInput")
    wv = nc.dram_tensor("wv", [HID, D], bf16, kind="ExternalInput")
    wo = nc.dram_tensor("wo", [HID, TB], bf16, kind="ExternalInput")
    cosE = nc.dram_tensor("cosE", [D, S], bf16, kind="ExternalInput")
    sinP = nc.dram_tensor("sinP", [D, S], bf16, kind="ExternalInput")
    pswap = nc.dram_tensor("pswap", [128, 128], bf16, kind="ExternalInput")
    ident = nc.dram_tensor("ident", [128, 128], bf16, kind="ExternalInput")
    ones_in = nc.dram_tensor("ones_in", [128, 1], bf16, kind="ExternalInput")
    mmask = nc.dram_tensor("mmask", [max(n_u, 1) * 128, TB], bf16,
                           kind="ExternalInput")
    out_o = nc.dram_tensor("o", [S, TB], f32, kind="ExternalOutput")

    mm = nc.tensor.matmul

    with tile.TileContext(nc, num_cores=N_CORES) as tc:
        stk0 = nc.allow_low_precision("bf16 kernel; fp32 PSUM accumulate")
        stk0.__enter__()
        with (
            tc.tile_pool(name="const", bufs=1) as cpool,
            tc.tile_pool(name="acts", bufs=1) as apool,
            tc.tile_pool(name="xs", bufs=2) as xpool,
            tc.tile_pool(name="gsp", bufs=2) as gsp,
            tc.tile_pool(name="ex", bufs=8) as epool,
            tc.tile_pool(name="st", bufs=2) as stage,
            tc.tile_pool(name="pj", bufs=3, space="PSUM") as pjp,
            tc.tile_pool(name="pa", bufs=3, space="PSUM") as pap,
            tc.tile_pool(name="pc", bufs=2, space="PSUM") as pcp,
            tc.tile_pool(name="dram", bufs=1, space="DRAM") as dram,
        ):
            # ---------- initial loads, ordered for fastest compute start ----
            xt_tiles = {}

            def load_xt(t):
                xt = xpool.tile([128, HT * TB], bf16, tag="xt", name=f"xt{t}")
                for hc in range(4):
                    nc.sync.dma_start(
                        out=xt[:, hc * 4 * TB:(hc + 1) * 4 * TB].rearrange(
                            "p (hh n) -> p hh n", n=TB),
                        in_=xT[hc * 512:(hc + 1) * 512,
                               t * TB:(t + 1) * TB].rearrange(
                            "(hh p) n -> p hh n", p=128),
                    )
                xt_tiles[t] = xt

            wq_s = cpool.tile([128, HT * HLOC * D], bf16, tag="wq")
            # interleave x(t=0) chunks with wq chunks so pass 1 starts early
            for hc in range(4):
                xt = xt_tiles.get(0)
                if xt is None:
                    xt = xpool.tile([128, HT * TB], bf16, tag="xt", name="xt0")
                    xt_tiles[0] = xt
                nc.sync.dma_start(
                    out=xt[:, hc * 4 * TB:(hc + 1) * 4 * TB].rearrange(
                        "p (hh n) -> p hh n", n=TB),
                    in_=xT[hc * 512:(hc + 1) * 512, 0:TB].rearrange(
                        "(hh p) n -> p hh n", p=128),
                )
                nc.sync.dma_start(
                    out=wq_s[:, hc * 4 * HLOC * D:(hc + 1) * 4 * HLOC * D]
                        .rearrange("p (hh n) -> p hh n", n=HLOC * D),
                    in_=wq[hc * 512:(hc + 1) * 512, :].rearrange(
                        "(hh p) n -> p hh n", p=128),
                )
            wk_s = cpool.tile([128, HT * D], bf16, tag="wk")
            nc.sync.dma_start(
                out=wk_s[:].rearrange("p (h n) -> p h n", n=D),
                in_=wk[:].rearrange("(h p) n -> p h n", p=128),
            )
            wv_s = cpool.tile([128, HT * D], bf16, tag="wv")
            nc.sync.dma_start(
                out=wv_s[:].rearrange("p (h n) -> p h n", n=D),
                in_=wv[:].rearrange("(h p) n -> p h n", p=128),
            )
            ps_s = cpool.tile([128, 128], bf16, tag="ps")
            nc.sync.dma_start(out=ps_s[:], in_=pswap[:])
            id_s = cpool.tile([128, 128], bf16, tag="id")
            nc.sync.dma_start(out=id_s[:], in_=ident[:])
            ones_s = cpool.tile([128, 1], bf16, tag="ones")
            nc.sync.dma_start(out=ones_s[:], in_=ones_in[:])
            cos_s = cpool.tile([D, S], bf16, tag="cos")
            nc.sync.dma_start(out=cos_s[:], in_=cosE[:])
            sin_s = cpool.tile([D, S], bf16, tag="sin")
            nc.sync.dma_start(out=sin_s[:], in_=sinP[:])
            mm_s = None
            if n_u:
                mm_s = cpool.tile([128, n_u * TB], bf16, tag="mm")
                nc.sync.dma_start(
                    out=mm_s[:].rearrange("p (u n) -> p u n", n=TB),
                    in_=mmask[:].rearrange("(u p) n -> p u n", p=128),
                )

            # persistent activations
            qT_s = apool.tile([128, HLOC * S], bf16, tag="qT")
            kT_s = apool.tile([128, S], bf16, tag="kT")
            v_s = apool.tile([128, S], bf16, tag="v")
            ctxT_s = apool.tile([128, HLOC * S], bf16, tag="ctxT")
            wo_s = apool.tile([128, HT * TB], bf16, tag="wo")

            bounce = [dram.tile([128, HLOC * TB], bf16, tag=f"bn{qb}",
                                name=f"bounce{qb}") for qb in range(NTB)]
            gath = [dram.tile([HLOC * 128, HLOC * TB], bf16, tag=f"g{qb}",
                              name=f"gath{qb}") for qb in range(NTB)]
            gs_tiles = {}

            # ---------------- building blocks ----------------
            def emit_proj(t):
                """QKV projection + RoPE + V transpose for token block t."""
                xt = xt_tiles[t]
                if t + 1 < NTB:
                    load_xt(t + 1)
                # weight column selectors: (sbuf, col_offset_fn) per output
                outs = []  # (psum_tile, kind, idx)
                passes = [[("q", 0), ("q", 1)], [("q", 2), ("q", 3)],
                          [("k", 0), ("v", 0)]]
                drains = []
                for pa_ in passes:
                    ptiles = []
                    for kind, i in pa_:
                        pt = pjp.tile([128, TB], f32, tag="pj",
                                      name=f"pj_{t}_{kind}{i}")
                        ptiles.append((pt, kind, i))
                    for h in range(HT):
                        xs = xt[:, h * TB:(h + 1) * TB]
                        st_, sp_ = (h == 0), (h == HT - 1)
                        for pt, kind, i in ptiles:
                            if kind == "q":
                                w = wq_s[:, h * HLOC * D + i * D:
                                         h * HLOC * D + (i + 1) * D]
                            elif kind == "k":
                                w = wk_s[:, h * D:(h + 1) * D]
                            else:
                                w = wv_s[:, h * D:(h + 1) * D]
                            mm(pt[:], w, xs, start=st_, stop=sp_)
                    # drain this pass (frees the pj slots for the next pass)
                    for pt, kind, i in ptiles:
                        if kind == "q":
                            sl = qT_s[:, i * S + t * TB: i * S + (t + 1) * TB]
                            nc.scalar.copy(sl, pt[:])
                            drains.append(sl)
                        elif kind == "k":
                            sl = kT_s[:, t * TB:(t + 1) * TB]
                            nc.scalar.copy(sl, pt[:])
                            drains.append(sl)
                        else:
                            vstg = stage.tile([128, TB], bf16, tag="vstg")
                            nc.vector.tensor_copy(vstg[:], pt[:])
                            outs.append(vstg)
                # RoPE on the 5 q/k chunks (in place in SBUF)
                for ch in drains:
                    sw = pap.tile([128, TB], f32, tag="aux", name="sw")
                    mm(sw[:], ps_s[:], ch, start=True, stop=True)
                    swm = stage.tile([128, TB], bf16, tag="swm")
                    nc.vector.tensor_mul(swm[:], sw[:],
                                         sin_s[:, t * TB:(t + 1) * TB])
                    nc.vector.tensor_mul(ch, ch, cos_s[:, t * TB:(t + 1) * TB])
                    nc.vector.tensor_add(ch, ch, swm[:])
                # V transpose into v_s
                vstg = outs[0]
                for i in range(TB // 128):
                    tps = pap.tile([128, 128], bf16, tag="aux", name="tps")
                    nc.tensor.transpose(tps[:], vstg[:, i * 128:(i + 1) * 128],
                                        id_s[:])
                    tt = t * (TB // 128) + i
                    nc.vector.tensor_copy(v_s[:, tt * 128:(tt + 1) * 128],
                                          tps[:])

            def emit_attn(qb):
                """Attention for all 4 local heads on q block qb."""
                lv = live[qb]
                diag = [kt for kt in lv if mixd[(qb, kt)] is not None]
                full = [kt for kt in lv if mixd[(qb, kt)] is None]
                # S issue order: couple of fulls first (they only depend on
                # old rounds), then diags (freshly roped + need mask), then
                # the rest. A consumption order: fulls first, diags last.
                s_order = full[:2] + diag + full[2:]
                a_order = full + diag
                s_pos = {kt: i for i, kt in enumerate(s_order)}
                L = len(lv)
                nquad = (L + 3) // 4
                DEPTH = 2
                for h in range(HLOC):
                    qslice = qT_s[:, h * S + qb * TB: h * S + (qb + 1) * TB]
                    cps = pcp.tile([128, TB], f32, tag="acc",
                                   name=f"cps{h}")
                    dps = pjp.tile([1, TB], f32, tag="pj", name=f"dps{h}")
                    ex_map = {}
                    state = {"pend": [], "quad": 0, "nadd": 0}

                    def consume(kt, a_idx):
                        ex = ex_map[kt]
                        st_, sp_ = (a_idx == 0), (a_idx == L - 1)
                        mm(cps[:], v_s[:, kt * 128:(kt + 1) * 128], ex[:],
                           start=st_, stop=sp_)
                        state["pend"].append(ex)
                        if len(state["pend"]) == 2:
                            exs = epool.tile([128, TB], bf16, tag="exs")
                            eng = (nc.vector if state["nadd"] % 2 == 0
                                   else nc.gpsimd)
                            eng.tensor_add(exs[:], state["pend"][0][:],
                                           state["pend"][1][:])
                            state["nadd"] += 1
                            state["pend"] = [exs]
                        if a_idx % 4 == 3 or sp_:
                            mm(dps[:], ones_s[:], state["pend"][0][:],
                               start=(state["quad"] == 0),
                               stop=(state["quad"] == nquad - 1))
                            state["quad"] += 1
                            state["pend"] = []

                    ai = 0
                    for si, kt in enumerate(s_order):
                        sps = pap.tile([128, TB], f32, tag="aux", name="sps")
                        mm(sps[:], kT_s[:, kt * 128:(kt + 1) * 128], qslice,
                           start=True, stop=True)
                        ex = epool.tile([128, TB], bf16, tag="ex")
                        nc.scalar.activation(ex[:], sps[:], EXP, scale=SCALE)
                        u = mixd[(qb, kt)]
                        if u is not None:
                            nc.vector.tensor_mul(ex[:], ex[:],
                                                 mm_s[:, u * TB:(u + 1) * TB])
                        ex_map[kt] = ex
                        # consume lagging A's whose ex has ≥DEPTH mm of slack
                        while ai < L and s_pos[a_order[ai]] <= si - DEPTH:
                            consume(a_order[ai], ai)
                            ai += 1
                    while ai < L:
                        consume(a_order[ai], ai)
                        ai += 1
                    # normalize: recip of denominator, broadcast, scale ctx
                    rc = stage.tile([1, TB], f32, tag="rc")
                    nc.vector.reciprocal_approx_fast(rc[:], dps[:])
                    rcb = stage.tile([1, TB], bf16, tag="rcb")
                    nc.vector.tensor_copy(rcb[:], rc[:])
                    rcbb = stage.tile([128, TB], bf16, tag="rcbb")
                    nc.gpsimd.partition_broadcast(rcbb[:], rcb[:])
                    nc.vector.tensor_mul(
                        ctxT_s[:, h * S + qb * TB: h * S + (qb + 1) * TB],
                        cps[:], rcbb[:])

            def emit_gather(qb):
                bc = bounce[qb]
                for h in range(HLOC):
                    nc.sync.dma_start(
                        out=bc[:, h * TB:(h + 1) * TB],
                        in_=ctxT_s[:, h * S + qb * TB: h * S + (qb + 1) * TB])
                nc.gpsimd.collective_compute(
                    "AllGather",
                    mybir.AluOpType.bypass,
                    replica_groups=GROUPS,
                    ins=[bc.opt()],
                    outs=[gath[qb].opt()],
                )
                gs = gsp.tile([128, HLOC * HLOC * TB], bf16, tag="gs",
                              name=f"gs{qb}")
                nc.sync.dma_start(
                    out=gs[:].rearrange("p (j n) -> p j n", n=HLOC * TB),
                    in_=gath[qb][:].rearrange("(j p) n -> p j n", p=128),
                )
                gs_tiles[qb] = gs

            def emit_outproj(qb):
                gs = gs_tiles[qb]
                for i in range(4):
                    ops = pcp.tile([128, TB], f32, tag="acc", name=f"ops{i}")
                    for g in range(HT):
                        j, h = g // HLOC, g % HLOC
                        mm(ops[:],
                           gs[:, j * HLOC * TB + h * TB + i * 128:
                              j * HLOC * TB + h * TB + (i + 1) * 128],
                           wo_s[:, g * TB:(g + 1) * TB],
                           start=(g == 0), stop=(g == HT - 1))
                    osb = stage.tile([128, TB], f32, tag="osb")
                    nc.vector.tensor_copy(osb[:], ops[:])
                    tt = qb * 4 + i
                    nc.sync.dma_start(out=out_o[tt * 128:(tt + 1) * 128, :],
                                      in_=osb[:])

            # ---------------- the interleaved schedule ----------------
            emit_proj(0)
            emit_attn(0)
            emit_gather(0)
            # wo load: needed first by outproj(0) in round 2
            nc.sync.dma_start(
                out=wo_s[:].rearrange("p (h n) -> p h n", n=TB),
                in_=wo[:].rearrange("(h p) n -> p h n", p=128),
            )
            emit_proj(1)
            emit_attn(1)
            emit_gather(1)
            emit_proj(2)
            emit_outproj(0)
            emit_attn(2)
            emit_gather(2)
            emit_proj(3)
            emit_outproj(1)
            emit_attn(3)
            emit_gather(3)
            emit_outproj(2)
            emit_outproj(3)
        stk0.__exit__(None, None, None)
    nc.compile()
    return nc


def kernel(x, wq, wk, wv, wo, cos, sin, mask):
    global LAST_RESULTS
    import ml_dtypes
    from concourse.bass_utils import run_bass_kernel_spmd

    bfnp = ml_dtypes.bfloat16
    x = np.asarray(x, np.float32)
    wq = np.asarray(wq, np.float32)
    wk = np.asarray(wk, np.float32)
    wv = np.asarray(wv, np.float32)
    wo = np.asarray(wo, np.float32)
    cos = np.asarray(cos, np.float32)
    sin = np.asarray(sin, np.float32)

    live, mixd, uniq = _analyze_mask(mask)
    n_u = len(uniq)
    mmask = (np.concatenate(uniq, axis=0) if n_u
             else np.zeros((128, TB), np.float32))

    cosE = np.repeat(cos, 2, axis=1).T
    sp = np.repeat(sin, 2, axis=1).copy()
    sp[:, 0::2] *= -1.0
    sinP = sp.T
    pswap = np.zeros((128, 128), np.float32)
    pswap[np.arange(128), np.arange(128) ^ 1] = 1.0
    ident = np.eye(128, dtype=np.float32)

    nc = _build_program(live, mixd, n_u)

    def b(a):
        return np.ascontiguousarray(np.asarray(a).astype(bfnp))

    in_maps = []
    for c in range(N_CORES):
        bb, j = c // 4, c % 4
        in_maps.append({
            "xT": b(x[bb].T),
            "wq": b(wq[:, 512 * j:512 * (j + 1)]),
            "wk": b(wk[:, 128 * j:128 * (j + 1)]),
            "wv": b(wv[:, 128 * j:128 * (j + 1)]),
            "wo": b(wo[:, 512 * j:512 * (j + 1)]),
            "cosE": b(cosE), "sinP": b(sinP), "pswap": b(pswap),
            "ident": b(ident),
            "ones_in": b(np.ones((128, 1), np.float32)),
            "mmask": b(mmask),
        })

    res = run_bass_kernel_spmd(nc, in_maps, list(range(N_CORES)))
    LAST_RESULTS = res

    out = np.empty((B, S, HID), np.float32)
    for c in range(N_CORES):
        bb, j = c // 4, c % 4
        out[bb, :, 512 * j:512 * (j + 1)] = res.results[c]["o"]
    return out
